# revision 31
# baseline (speedup 1.0000x reference)
"""Trainium2 Bass kernel for batched single-head attention with QKV projections.

Reference computation (B=4, Lq=Lk=2048, Dm=1024, Dk=Dv=128):
    q = Q @ WQ + bQ ; k = K @ WK + bK ; v = V @ WV + bV
    out = softmax(q k^T / sqrt(Dk)) v

Sharding: 8 cores; core c handles batch b=c//2, query half h=c%2
(1024 queries per core). K/V *projection* for the batch is split across
the pair: each core projects only its own key half (1024 keys), then the
pair exchanges projected kT / v via SBUF->SBUF remote_dma_broadcast
(partner = tpb XOR 1, expressed with relative dests so the SPMD program
is symmetric). Keys are indexed local-half-first on every core; softmax
is permutation-invariant over keys so no global ordering is needed.
This halves both the HBM input traffic (6.85MB vs 10.8MB per core) and
the K/V projection FLOPs versus full replication.

Inputs stream over one HWDGE FIFO queue in compute-consumption order
(wq,wk -> qt0 -> kt0 -> qt1 -> kt1 -> wv -> vt0 -> vt1) so the PE
pipeline starts ~3us after the first byte and never starves.

Softmax is computed without max-subtraction (scores ~ N(0,1), max over
8M samples ~ 5.7 sigma -> exp <= ~300, safely in range): scoresT[k,q]
tiles come out of the PE, ScalarE applies exp(scale*x) straight from
PSUM into bf16 SBUF tiles, and the denominator falls out of the AV
matmul via a ones-column planted in v by a rank-1 bias matmul.

Cross-core sync: Tile's scheduling sim cannot model the partner's
remote-sem increments, so the recv waits are attached post-schedule to
Vector-engine "carrier" memsets that the remote-consuming matmuls
depend on through normal Tile edges. Carriers are ordered after our own
trigger_dma so the pair cannot deadlock.
"""

import os
import sys

sys.path.insert(0, "/opt/trn_rl_repo")

import numpy as np
import ml_dtypes

import concourse.bass as bass
import concourse.bacc as bacc
import concourse.tile as tile
import concourse.mybir as mybir
from concourse import library_config
from concourse import bass2jax as _b2j
from concourse.bass_utils import run_bass_kernel_spmd


def _run_bass_via_pjrt_synced(nc, in_maps, n_cores):
    """Drop-in replacement for bass2jax.run_bass_via_pjrt (multi-core) that
    device_put()s every core's inputs and BLOCKS before launching, so the 8
    NEFF executions start near-simultaneously. Without this, per-device h2d
    uploads serialize and stagger core starts by ~100s of us, which any
    cross-core semaphore wait then eats.
    """
    import jax
    from jax.sharding import Mesh, PartitionSpec, NamedSharding
    from jax.experimental.shard_map import shard_map

    if n_cores == 1 or nc.dbg_addr is not None:
        return _ORIG_RUN_VIA_PJRT(nc, in_maps, n_cores)
    _b2j.install_neuronx_cc_hook()

    partition_name = nc.partition_id_tensor.name if nc.partition_id_tensor else None
    in_names, out_names, out_avals, zero_outs = [], [], [], []
    for alloc in nc.m.functions[0].allocations:
        if not isinstance(alloc, mybir.MemoryLocationSet):
            continue
        name = alloc.memorylocations[0].name
        if alloc.kind == "ExternalInput":
            if name != partition_name:
                in_names.append(name)
        elif alloc.kind == "ExternalOutput":
            shape = tuple(alloc.tensor_shape)
            dtype = mybir.dt.np(alloc.dtype)
            out_names.append(name)
            out_avals.append(jax.core.ShapedArray(shape, dtype))
            zero_outs.append(np.zeros(shape, dtype))
    n_params = len(in_names)
    n_outs = len(out_avals)
    in_names.extend(out_names)
    if partition_name is not None:
        in_names.append(partition_name)
    donate = tuple(range(n_params, n_params + n_outs))

    def _body(*args):
        operands = list(args)
        if partition_name is not None:
            operands.append(_b2j.partition_id_tensor())
        outs = _b2j._bass_exec_p.bind(
            *operands,
            out_avals=tuple(out_avals),
            in_names=tuple(in_names),
            out_names=tuple(out_names),
            lowering_input_output_aliases=(),
            sim_require_finite=True,
            sim_require_nnan=True,
            nc=nc,
        )
        return tuple(outs)

    devices = jax.devices()[:n_cores]
    mesh = Mesh(np.asarray(devices), ("core",))
    in_specs = (PartitionSpec("core"),) * (n_params + n_outs)
    out_specs = (PartitionSpec("core"),) * len(out_names)
    sharded = jax.jit(
        shard_map(_body, mesh=mesh, in_specs=in_specs, out_specs=out_specs,
                  check_rep=False),
        donate_argnums=donate, keep_unused=True)

    per_core = [[np.asarray(m[name]) for name in in_names[:n_params]]
                for m in in_maps]
    ns = NamedSharding(mesh, PartitionSpec("core"))
    dev_args = [
        jax.device_put(
            np.concatenate([per_core[c][i] for c in range(n_cores)], axis=0), ns)
        for i in range(n_params)
    ] + [
        jax.device_put(
            np.zeros((n_cores * z.shape[0], *z.shape[1:]), z.dtype), ns)
        for z in zero_outs
    ]
    jax.block_until_ready(dev_args)
    out_arrs = sharded(*dev_args)
    jax.block_until_ready(out_arrs)
    return [
        {name: np.asarray(out_arrs[i]).reshape(n_cores, *out_avals[i].shape)[c]
         for i, name in enumerate(out_names)}
        for c in range(n_cores)
    ]


_ORIG_RUN_VIA_PJRT = _b2j.run_bass_via_pjrt
_b2j.run_bass_via_pjrt = _run_bass_via_pjrt_synced

BF16 = ml_dtypes.bfloat16

B, LQ, LK, DM, DK, DV = 4, 2048, 2048, 1024, 128, 128
N_CORES = 8
LQ_C = LQ // 2          # queries per core
LK_H = LK // 2          # keys projected locally (half)
N_DM = DM // 128        # dm blocks
N_KB = LK // 128        # key blocks total (local-first indexing)
N_KB_H = N_KB // 2      # key blocks per half
N_QB = LQ_C // 128      # query blocks per core
SCALE = 1.0 / float(np.sqrt(DK))

_CACHED_NC = None
LAST_EXEC_NS = None


def _build():
    dt = mybir.dt
    nc = bacc.Bacc("TRN2", target_bir_lowering=False, debug=False,
                   num_devices=N_CORES, num_swdge_queues=2)

    wq_d = nc.dram_tensor("wq", [128, N_DM, DK], dt.bfloat16, kind="ExternalInput")
    wk_d = nc.dram_tensor("wk", [128, N_DM, DK], dt.bfloat16, kind="ExternalInput")
    wv_d = nc.dram_tensor("wv", [128, N_DM, DV], dt.bfloat16, kind="ExternalInput")
    qt_d = nc.dram_tensor("qt", [2, 128, N_DM, 512], dt.bfloat16, kind="ExternalInput")
    kt_d = nc.dram_tensor("kt", [2, 128, N_DM, 512], dt.bfloat16, kind="ExternalInput")
    vt_d = nc.dram_tensor("vt", [2, 128, N_DM, 512], dt.bfloat16, kind="ExternalInput")
    b2_d = nc.dram_tensor("b2", [DK, 2], dt.float32, kind="ExternalInput")
    bva_d = nc.dram_tensor("bvaug", [1, DV + 1], dt.bfloat16, kind="ExternalInput")
    out_d = nc.dram_tensor("out", [128, N_QB, DV], dt.float32, kind="ExternalOutput")

    with tile.TileContext(nc) as tc:
        with tc.tile_pool(name="sb", bufs=1) as sb:
            # --- resident SBUF tensors ---
            wq_sb = sb.tile([128, N_DM, DK], dt.bfloat16)
            wk_sb = sb.tile([128, N_DM, DK], dt.bfloat16)
            wv_sb = sb.tile([128, N_DM, DV], dt.bfloat16)
            b2 = sb.tile([DK, 2], dt.float32)
            bva = sb.tile([1, DV + 1], dt.bfloat16)
            ones = sb.tile([1, 128], dt.bfloat16)
            warm = sb.tile([128, 256], dt.bfloat16)
            qt_sb = sb.tile([128, 2, N_DM, 512], dt.bfloat16)
            kt_sb = sb.tile([128, 2, N_DM, 512], dt.bfloat16)
            vt_sb = sb.tile([128, 2, N_DM, 512], dt.bfloat16)
            qT = sb.tile([DK, 2, 512], dt.bfloat16)         # projected q
            kT_loc = sb.tile([DK, LK_H], dt.bfloat16)       # own projected k half
            kT_rem = sb.tile([DK, LK_H], dt.bfloat16)       # partner's k half
            v_loc = sb.tile([128, N_KB_H, DV + 1], dt.bfloat16)
            v_rem = sb.tile([128, N_KB_H, DV + 1], dt.bfloat16)
            pT = sb.tile([128, N_KB, 2, 512], dt.bfloat16)  # exp scores [k, q]
            out_sb = sb.tile([128, N_QB, DV], dt.float32)
            recip = sb.tile([128, N_QB, 1], dt.float32)
            car_kt = sb.tile([1, 1], dt.float32)
            car_vt = sb.tile([1, 1], dt.float32)

            rs_k = nc.alloc_semaphore("rs_k")
            rs_v = nc.alloc_semaphore("rs_v")
            ls = nc.alloc_semaphore("rdma_local")

            nc.scalar.dma_start(b2[:], b2_d.ap())
            nc.scalar.dma_start(bva[:], bva_d.ap())
            nc.vector.memset(ones[:], 1.0)
            nc.vector.memset(warm[:], 1.0)
            libload = nc.gpsimd.load_library(library_config.remote_dma)

            # Input stream: one HWDGE engine queue -> FIFO arrival in
            # exactly this order, matching compute consumption.
            nc.sync.dma_start(wq_sb[:], wq_d.ap())
            nc.sync.dma_start(wk_sb[:], wk_d.ap())
            nc.sync.dma_start(qt_sb[:, 0, :, :], qt_d.ap()[0])
            nc.sync.dma_start(kt_sb[:, 0, :, :], kt_d.ap()[0])
            nc.sync.dma_start(kt_sb[:, 1, :, :], kt_d.ap()[1])
            nc.sync.dma_start(qt_sb[:, 1, :, :], qt_d.ap()[1])
            nc.sync.dma_start(wv_sb[:], wv_d.ap())
            nc.sync.dma_start(vt_sb[:, 0, :, :], vt_d.ap()[0])
            nc.sync.dma_start(vt_sb[:, 1, :, :], vt_d.ap()[1])

            with tc.tile_pool(name="ps_o", bufs=3, space="PSUM") as ps_o, \
                 tc.tile_pool(name="ps_s", bufs=3, space="PSUM") as ps_s:
                pso = [ps_o.tile([128, 3, DV + 1], dt.float32, tag="pso",
                                 name=f"pso{j}") for j in range(3)]

                # --- HAM warmup: keep PE busy while inputs stream in ---
                with tc.tile_pool(name="ps_w", bufs=2, space="PSUM") as ps_w:
                    psw = ps_w.tile([128, 512], dt.float32, tag="psw", name="psw")
                    for _ in range(28):
                        nc.tensor.matmul(psw[:, 0:256], warm[:, 0:128], warm[:],
                                         start=True, stop=True)

                    # --- projections (q, k-half) share the ps_w slots ---
                    kT_writers = []

                    def qproj(nt):
                        psq = ps_w.tile([128, 512], dt.float32, name=f"psq{nt}",
                                        tag="psw")
                        for i in range(N_DM):
                            nc.tensor.matmul(
                                psq[:], wq_sb[:, i, :], qt_sb[:, nt, i, :],
                                start=(i == 0), stop=(i == N_DM - 1))
                        nc.vector.tensor_scalar_add(qT[:, nt, :], psq[:],
                                                    b2[:, 0:1])

                    def kproj(ch):
                        psk = ps_w.tile([128, 512], dt.float32, name=f"psk{ch}",
                                        tag="psw")
                        for i in range(N_DM):
                            nc.tensor.matmul(
                                psk[:], wk_sb[:, i, :], kt_sb[:, ch, i, :],
                                start=(i == 0), stop=(i == N_DM - 1))
                        kT_writers.append(nc.vector.tensor_scalar_add(
                            kT_loc[:, ch * 512:(ch + 1) * 512], psk[:],
                            b2[:, 1:2]))

                    def scores(kb, nts, kt_buf, kb_off):
                        # kb is the local-first block index into pT
                        for nt in nts:
                            pss = ps_s.tile([128, 512], dt.float32, name="pss",
                                            tag="pss")
                            nc.tensor.matmul(
                                pss[:],
                                kt_buf[:, (kb - kb_off) * 128:
                                       (kb - kb_off + 1) * 128],
                                qT[:, nt, :], start=True, stop=True)
                            nc.scalar.activation(
                                pT[:, kb, nt, :], pss[:],
                                mybir.ActivationFunctionType.Exp, scale=SCALE)

                    # kproj(1) right after the first scores batch (and kt1
                    # streamed before qt1) so the kT send fires ~20us instead
                    # of ~30us -- the partner's half then lands before the
                    # remote-score matmuls reach the head of the PE queue.
                    qproj(0)
                    kproj(0)
                    for kb in range(0, 4):
                        scores(kb, [0], kT_loc, 0)
                    kproj(1)
                    # kb4-7 nt0 before the nt1 batch: their inputs (kT c1,
                    # qT[0]) are ready, so the exp stream doesn't pause while
                    # qproj(1) waits for qt1 to arrive.
                    for kb in range(4, N_KB_H):
                        scores(kb, [0], kT_loc, 0)
                    qproj(1)
                    for kb in range(0, N_KB_H):
                        scores(kb, [1], kT_loc, 0)

                # --- send projected k half to the pair partner ---
                # The 8-slot broadcast drains through ~1 SDMA lane (~25GB/s,
                # ~10.5us for the full kT) -- the systematic 9us carrier lag.
                # Split kT across TWO SWDGE queues so the halves transfer in
                # parallel rings, each fired as soon as its kproj chunk lands.
                prep_ks = []
                for ci in range(2):
                    p = nc.gpsimd.remote_dma_broadcast(
                        kT_rem[:, ci * 512:(ci + 1) * 512],
                        kT_loc[:, ci * 512:(ci + 1) * 512], rs_k, ls,
                        rdests=[(0, 1)] + [None] * 7, queue_num=ci)
                    bass._add_dep_helper(p.ins, libload.ins, sync=True,
                                         reason="library before desc-gen")
                    prep_ks.append(p)
                trig_k = None
                for ci in range(2):
                    t = nc.gpsimd.trigger_dma(queue_num=ci)
                    bass._add_dep_helper(t.ins, prep_ks[ci].ins, sync=True,
                                         reason="prep before trigger")
                    bass._add_dep_helper(t.ins, kT_writers[ci].ins, sync=True,
                                         reason="kT chunk before send")
                    trig_k = t

                # --- v projection (local half) ---
                with tc.tile_pool(name="ps_v", bufs=2, space="PSUM") as ps_v:
                    v_writers = []
                    for kb in range(N_KB_H):
                        psv = ps_v.tile([128, DV + 1], dt.float32, name="psv",
                                        tag="psv")
                        nc.tensor.matmul(psv[:], ones[:1, :], bva[:1, :],
                                         start=True, stop=False)
                        for i in range(N_DM):
                            nc.tensor.matmul(
                                psv[:, 0:DV],
                                vt_sb[:, kb // 4, i,
                                      (kb % 4) * 128:(kb % 4 + 1) * 128],
                                wv_sb[:, i, :],
                                start=False, stop=(i == N_DM - 1))
                        v_writers.append(
                            nc.vector.tensor_copy(v_loc[:, kb, :], psv[:]))

                    prep_v = nc.gpsimd.remote_dma_broadcast(
                        v_rem[:], v_loc[:], rs_v, ls,
                        rdests=[(0, 1)] + [None] * 7)
                    bass._add_dep_helper(prep_v.ins, prep_ks[0].ins, sync=True,
                                         reason="queue-0 ring FIFO order")
                    trig_v = nc.gpsimd.trigger_dma()
                    bass._add_dep_helper(trig_v.ins, prep_v.ins, sync=True,
                                         reason="prep before trigger")
                    bass._add_dep_helper(trig_v.ins, trig_k.ins, sync=True,
                                         reason="trigger order")
                    for w in v_writers:
                        bass._add_dep_helper(trig_v.ins, w.ins, sync=True,
                                             reason="v data before send")

                    # --- AV over the local half (k-major accumulate) ---
                    def av(kb, v_buf, kb_off, gate=None):
                        for qb in range(N_QB):
                            mm = nc.tensor.matmul(
                                pso[qb // 3][:, qb % 3, :],
                                pT[:, kb, qb // 4,
                                   (qb % 4) * 128:(qb % 4 + 1) * 128],
                                v_buf[:, kb - kb_off, :],
                                start=(kb == 0 and qb % 3 == 0),
                                stop=(kb == N_KB - 1),
                                skip_group_check=True)
                            if gate is not None:
                                bass._add_dep_helper(mm.ins, gate.ins,
                                                     sync=True,
                                                     reason="remote data gate")

                    # carriers: stall until the partner's halves have landed.
                    # nosync (ordering-only) deps keep the carrier's inline
                    # wait slot free for the post-schedule remote-sem wait,
                    # while still pinning it behind every Vector op our own
                    # sends need (same-engine FIFO = hard ordering), so the
                    # pair cannot deadlock.
                    car_k = nc.vector.memset(car_kt[:], 0.0)
                    for w in kT_writers + v_writers:
                        bass._add_dep_helper(car_k.ins, w.ins, sync=False,
                                             reason="send data first")
                    car_v = nc.vector.memset(car_vt[:], 0.0)
                    bass._add_dep_helper(car_v.ins, car_k.ins, sync=False,
                                         reason="carrier order")

                    def scores_rem(kbs):
                        for kb in kbs:
                            for nt in range(2):
                                pss = ps_s.tile([128, 512], dt.float32,
                                                name="pss", tag="pss")
                                mm = nc.tensor.matmul(
                                    pss[:],
                                    kT_rem[:, (kb - N_KB_H) * 128:
                                           (kb - N_KB_H + 1) * 128],
                                    qT[:, nt, :], start=True, stop=True)
                                bass._add_dep_helper(mm.ins, car_k.ins,
                                                     sync=True,
                                                     reason="kT_rem gate")
                                nc.scalar.activation(
                                    pT[:, kb, nt, :], pss[:],
                                    mybir.ActivationFunctionType.Exp,
                                    scale=SCALE)

                    # Remote scores sandwiched between local AV batches: the
                    # ScalarE exp stream starts as soon as the partner's kT
                    # lands, while AV-local batches buffer the PE against a
                    # late carrier.
                    av(0, v_loc, 0)
                    av(1, v_loc, 0)
                    av(2, v_loc, 0)
                    av(3, v_loc, 0)
                    scores_rem(range(8, 12))
                    av(4, v_loc, 0)
                    av(5, v_loc, 0)
                    scores_rem(range(12, 16))
                    av(6, v_loc, 0)
                    av(7, v_loc, 0)
                    for kb in range(N_KB_H, N_KB):
                        av(kb, v_rem, N_KB_H, gate=car_v)

                # --- normalize + output ---
                for qb in range(N_QB):
                    nc.vector.reciprocal(recip[:, qb, :],
                                         pso[qb // 3][:, qb % 3, DV:DV + 1])
                    if qb % 2 == 0:
                        nc.scalar.activation(
                            out_sb[:, qb, :], pso[qb // 3][:, qb % 3, 0:DV],
                            mybir.ActivationFunctionType.Copy,
                            scale=recip[:, qb, :])
                    else:
                        nc.vector.tensor_scalar_mul(
                            out_sb[:, qb, :], pso[qb // 3][:, qb % 3, 0:DV],
                            recip[:, qb, :])
                    if qb % 4 == 3:
                        nc.scalar.dma_start(
                            out_d.ap()[:, qb - 3:qb + 1, :],
                            out_sb[:, qb - 3:qb + 1, :])

    # Tile's scheduling sim can't see the partner's sem increments; attach
    # the real cross-core waits only after scheduling.
    # 8-slot broadcast with one live dest: remote_sem += 16 // 8 = 2
    # two kT chunk broadcasts, each +2
    car_k._wait_ge(rs_k, 4)
    car_v._wait_ge(rs_v, 2)
    nc.compile()
    return nc


def kernel(**inputs):
    global _CACHED_NC, LAST_EXEC_NS
    Q = np.asarray(inputs["Q"], dtype=np.float32)
    K = np.asarray(inputs["K"], dtype=np.float32)
    V = np.asarray(inputs["V"], dtype=np.float32)
    WQ = np.asarray(inputs["WQ"], dtype=np.float32)
    bQ = np.asarray(inputs["bQ"], dtype=np.float32)
    WK = np.asarray(inputs["WK"], dtype=np.float32)
    bK = np.asarray(inputs["bK"], dtype=np.float32)
    WV = np.asarray(inputs["WV"], dtype=np.float32)
    bV = np.asarray(inputs["bV"], dtype=np.float32)

    if _CACHED_NC is None:
        _CACHED_NC = _build()
    nc = _CACHED_NC

    def _w(M):  # [dm, dout] -> [128, n_dm, dout]
        return np.ascontiguousarray(
            M.reshape(N_DM, 128, M.shape[1]).transpose(1, 0, 2)).astype(BF16)

    wq = _w(WQ)
    wk = _w(WK)
    wv = _w(WV)
    b2 = np.ascontiguousarray(
        np.stack([bQ, bK], axis=1)).astype(np.float32)  # [DK, 2]
    bva = np.concatenate([bV, np.ones(1, np.float32)]).reshape(1, DV + 1).astype(BF16)

    def _blk(M):  # [1024 seq, dm] -> [2, 128, n_dm, 512] device layout
        return np.ascontiguousarray(
            M.T.reshape(N_DM, 128, 2, 512).transpose(2, 1, 0, 3)).astype(BF16)

    in_maps = []
    for c in range(N_CORES):
        b, h = c // 2, c % 2
        sl = slice(h * LK_H, (h + 1) * LK_H)
        in_maps.append({
            "qt": _blk(Q[b, h * LQ_C:(h + 1) * LQ_C, :]),
            "kt": _blk(K[b, sl, :]),
            "vt": _blk(V[b, sl, :]),
            "wq": wq, "wk": wk, "wv": wv, "b2": b2, "bvaug": bva,
        })

    trace = bool(os.environ.get("KERNEL_TRACE"))
    if trace:
        try:
            import axon_profile_shim  # noqa: F401
        except ImportError:
            trace = False

    res = run_bass_kernel_spmd(nc, in_maps, core_ids=list(range(N_CORES)),
                               trace=trace)
    LAST_EXEC_NS = res.exec_time_ns

    out = np.empty((B, LQ, DV), np.float32)
    for c in range(N_CORES):
        b, h = c // 2, c % 2
        blk = res.results[c]["out"]  # [128, N_QB, DV]
        out[b, h * LQ_C:(h + 1) * LQ_C, :] = (
            blk.transpose(1, 0, 2).reshape(LQ_C, DV))
    return out


# revision 33
# speedup vs baseline: 1.0470x; 1.0470x over previous
"""Trainium2 Bass kernel for batched single-head attention with QKV projections.

Reference computation (B=4, Lq=Lk=2048, Dm=1024, Dk=Dv=128):
    q = Q @ WQ + bQ ; k = K @ WK + bK ; v = V @ WV + bV
    out = softmax(q k^T / sqrt(Dk)) v

Sharding: 8 cores; core c handles batch b=c//2, query half h=c%2
(1024 queries per core). K/V *projection* for the batch is split across
the pair: each core projects only its own key half (1024 keys), then the
pair exchanges projected kT / v via SBUF->SBUF remote_dma_broadcast
(partner = tpb XOR 1, expressed with relative dests so the SPMD program
is symmetric). Keys are indexed local-half-first on every core; softmax
is permutation-invariant over keys so no global ordering is needed.
This halves both the HBM input traffic (6.85MB vs 10.8MB per core) and
the K/V projection FLOPs versus full replication.

Inputs stream over one HWDGE FIFO queue in compute-consumption order
(wq,wk -> qt0 -> kt0 -> qt1 -> kt1 -> wv -> vt0 -> vt1) so the PE
pipeline starts ~3us after the first byte and never starves.

Softmax is computed without max-subtraction (scores ~ N(0,1), max over
8M samples ~ 5.7 sigma -> exp <= ~300, safely in range): scoresT[k,q]
tiles come out of the PE, ScalarE applies exp(scale*x) straight from
PSUM into bf16 SBUF tiles, and the denominator falls out of the AV
matmul via a ones-column planted in v by a rank-1 bias matmul.

Cross-core sync: Tile's scheduling sim cannot model the partner's
remote-sem increments, so the recv waits are attached post-schedule to
Vector-engine "carrier" memsets that the remote-consuming matmuls
depend on through normal Tile edges. Carriers are ordered after our own
trigger_dma so the pair cannot deadlock.
"""

import os
import sys

sys.path.insert(0, "/opt/trn_rl_repo")

import numpy as np
import ml_dtypes

import concourse.bass as bass
import concourse.bacc as bacc
import concourse.tile as tile
import concourse.mybir as mybir
from concourse import library_config
from concourse import bass2jax as _b2j
from concourse.bass_utils import run_bass_kernel_spmd


def _run_bass_via_pjrt_synced(nc, in_maps, n_cores):
    """Drop-in replacement for bass2jax.run_bass_via_pjrt (multi-core) that
    device_put()s every core's inputs and BLOCKS before launching, so the 8
    NEFF executions start near-simultaneously. Without this, per-device h2d
    uploads serialize and stagger core starts by ~100s of us, which any
    cross-core semaphore wait then eats.
    """
    import jax
    from jax.sharding import Mesh, PartitionSpec, NamedSharding
    from jax.experimental.shard_map import shard_map

    if n_cores == 1 or nc.dbg_addr is not None:
        return _ORIG_RUN_VIA_PJRT(nc, in_maps, n_cores)
    _b2j.install_neuronx_cc_hook()

    partition_name = nc.partition_id_tensor.name if nc.partition_id_tensor else None
    in_names, out_names, out_avals, zero_outs = [], [], [], []
    for alloc in nc.m.functions[0].allocations:
        if not isinstance(alloc, mybir.MemoryLocationSet):
            continue
        name = alloc.memorylocations[0].name
        if alloc.kind == "ExternalInput":
            if name != partition_name:
                in_names.append(name)
        elif alloc.kind == "ExternalOutput":
            shape = tuple(alloc.tensor_shape)
            dtype = mybir.dt.np(alloc.dtype)
            out_names.append(name)
            out_avals.append(jax.core.ShapedArray(shape, dtype))
            zero_outs.append(np.zeros(shape, dtype))
    n_params = len(in_names)
    n_outs = len(out_avals)
    in_names.extend(out_names)
    if partition_name is not None:
        in_names.append(partition_name)
    donate = tuple(range(n_params, n_params + n_outs))

    def _body(*args):
        operands = list(args)
        if partition_name is not None:
            operands.append(_b2j.partition_id_tensor())
        outs = _b2j._bass_exec_p.bind(
            *operands,
            out_avals=tuple(out_avals),
            in_names=tuple(in_names),
            out_names=tuple(out_names),
            lowering_input_output_aliases=(),
            sim_require_finite=True,
            sim_require_nnan=True,
            nc=nc,
        )
        return tuple(outs)

    devices = jax.devices()[:n_cores]
    mesh = Mesh(np.asarray(devices), ("core",))
    in_specs = (PartitionSpec("core"),) * (n_params + n_outs)
    out_specs = (PartitionSpec("core"),) * len(out_names)
    sharded = jax.jit(
        shard_map(_body, mesh=mesh, in_specs=in_specs, out_specs=out_specs,
                  check_rep=False),
        donate_argnums=donate, keep_unused=True)

    per_core = [[np.asarray(m[name]) for name in in_names[:n_params]]
                for m in in_maps]
    ns = NamedSharding(mesh, PartitionSpec("core"))
    dev_args = [
        jax.device_put(
            np.concatenate([per_core[c][i] for c in range(n_cores)], axis=0), ns)
        for i in range(n_params)
    ] + [
        jax.device_put(
            np.zeros((n_cores * z.shape[0], *z.shape[1:]), z.dtype), ns)
        for z in zero_outs
    ]
    jax.block_until_ready(dev_args)
    out_arrs = sharded(*dev_args)
    jax.block_until_ready(out_arrs)
    return [
        {name: np.asarray(out_arrs[i]).reshape(n_cores, *out_avals[i].shape)[c]
         for i, name in enumerate(out_names)}
        for c in range(n_cores)
    ]


_ORIG_RUN_VIA_PJRT = _b2j.run_bass_via_pjrt
_b2j.run_bass_via_pjrt = _run_bass_via_pjrt_synced

BF16 = ml_dtypes.bfloat16

B, LQ, LK, DM, DK, DV = 4, 2048, 2048, 1024, 128, 128
N_CORES = 8
LQ_C = LQ // 2          # queries per core
LK_H = LK // 2          # keys projected locally (half)
N_DM = DM // 128        # dm blocks
N_KB = LK // 128        # key blocks total (local-first indexing)
N_KB_H = N_KB // 2      # key blocks per half
N_QB = LQ_C // 128      # query blocks per core
SCALE = 1.0 / float(np.sqrt(DK))

_CACHED_NC = None
LAST_EXEC_NS = None


def _build():
    dt = mybir.dt
    nc = bacc.Bacc("TRN2", target_bir_lowering=False, debug=False,
                   num_devices=N_CORES)

    wq_d = nc.dram_tensor("wq", [128, N_DM, DK], dt.bfloat16, kind="ExternalInput")
    wk_d = nc.dram_tensor("wk", [128, N_DM, DK], dt.bfloat16, kind="ExternalInput")
    wv_d = nc.dram_tensor("wv", [128, N_DM, DV], dt.bfloat16, kind="ExternalInput")
    qt_d = nc.dram_tensor("qt", [2, 128, N_DM, 512], dt.bfloat16, kind="ExternalInput")
    kt_d = nc.dram_tensor("kt", [2, 128, N_DM, 512], dt.bfloat16, kind="ExternalInput")
    vt_d = nc.dram_tensor("vt", [2, 128, N_DM, 512], dt.bfloat16, kind="ExternalInput")
    b2_d = nc.dram_tensor("b2", [DK, 2], dt.float32, kind="ExternalInput")
    bva_d = nc.dram_tensor("bvaug", [1, DV + 1], dt.bfloat16, kind="ExternalInput")
    out_d = nc.dram_tensor("out", [128, N_QB, DV], dt.float32, kind="ExternalOutput")

    with tile.TileContext(nc) as tc:
        with tc.tile_pool(name="sb", bufs=1) as sb:
            # --- resident SBUF tensors ---
            wq_sb = sb.tile([128, N_DM, DK], dt.bfloat16)
            wk_sb = sb.tile([128, N_DM, DK], dt.bfloat16)
            wv_sb = sb.tile([128, N_DM, DV], dt.bfloat16)
            b2 = sb.tile([DK, 2], dt.float32)
            bva = sb.tile([1, DV + 1], dt.bfloat16)
            ones = sb.tile([1, 128], dt.bfloat16)
            warm = sb.tile([128, 256], dt.bfloat16)
            qt_sb = sb.tile([128, 2, N_DM, 512], dt.bfloat16)
            kt_sb = sb.tile([128, 2, N_DM, 512], dt.bfloat16)
            vt_sb = sb.tile([128, 2, N_DM, 512], dt.bfloat16)
            qT = sb.tile([DK, 2, 512], dt.bfloat16)         # projected q
            kT_loc = sb.tile([DK, LK_H], dt.bfloat16)       # own projected k half
            kT_rem = sb.tile([DK, LK_H], dt.bfloat16)       # partner's k half
            v_loc = sb.tile([128, N_KB_H, DV + 1], dt.bfloat16)
            v_rem = sb.tile([128, N_KB_H, DV + 1], dt.bfloat16)
            pT = sb.tile([128, N_KB, 2, 512], dt.bfloat16)  # exp scores [k, q]
            out_sb = sb.tile([128, N_QB, DV], dt.float32)
            recip = sb.tile([128, N_QB, 1], dt.float32)
            car_kt = sb.tile([1, 1], dt.float32)
            car_kt2 = sb.tile([1, 1], dt.float32)
            car_vt = sb.tile([1, 1], dt.float32)

            rs_k = nc.alloc_semaphore("rs_k")
            rs_v = nc.alloc_semaphore("rs_v")
            ls = nc.alloc_semaphore("rdma_local")

            nc.scalar.dma_start(b2[:], b2_d.ap())
            nc.scalar.dma_start(bva[:], bva_d.ap())
            nc.vector.memset(ones[:], 1.0)
            nc.vector.memset(warm[:], 1.0)
            libload = nc.gpsimd.load_library(library_config.remote_dma)

            # Input stream: one HWDGE engine queue -> FIFO arrival in
            # exactly this order, matching compute consumption.
            nc.sync.dma_start(wq_sb[:], wq_d.ap())
            nc.sync.dma_start(wk_sb[:], wk_d.ap())
            nc.sync.dma_start(qt_sb[:, 0, :, :], qt_d.ap()[0])
            nc.sync.dma_start(kt_sb[:, 0, :, :], kt_d.ap()[0])
            nc.sync.dma_start(kt_sb[:, 1, :, :], kt_d.ap()[1])
            nc.sync.dma_start(qt_sb[:, 1, :, :], qt_d.ap()[1])
            nc.sync.dma_start(wv_sb[:], wv_d.ap())
            nc.sync.dma_start(vt_sb[:, 0, :, :], vt_d.ap()[0])
            nc.sync.dma_start(vt_sb[:, 1, :, :], vt_d.ap()[1])

            with tc.tile_pool(name="ps_o", bufs=3, space="PSUM") as ps_o, \
                 tc.tile_pool(name="ps_s", bufs=3, space="PSUM") as ps_s:
                pso = [ps_o.tile([128, 3, DV + 1], dt.float32, tag="pso",
                                 name=f"pso{j}") for j in range(3)]

                # --- HAM warmup: keep PE busy while inputs stream in ---
                with tc.tile_pool(name="ps_w", bufs=2, space="PSUM") as ps_w:
                    psw = ps_w.tile([128, 512], dt.float32, tag="psw", name="psw")
                    for _ in range(28):
                        nc.tensor.matmul(psw[:, 0:256], warm[:, 0:128], warm[:],
                                         start=True, stop=True)

                    # --- projections (q, k-half) share the ps_w slots ---
                    kT_writers = []

                    def qproj(nt):
                        psq = ps_w.tile([128, 512], dt.float32, name=f"psq{nt}",
                                        tag="psw")
                        for i in range(N_DM):
                            nc.tensor.matmul(
                                psq[:], wq_sb[:, i, :], qt_sb[:, nt, i, :],
                                start=(i == 0), stop=(i == N_DM - 1))
                        nc.vector.tensor_scalar_add(qT[:, nt, :], psq[:],
                                                    b2[:, 0:1])

                    def kproj(ch):
                        psk = ps_w.tile([128, 512], dt.float32, name=f"psk{ch}",
                                        tag="psw")
                        for i in range(N_DM):
                            nc.tensor.matmul(
                                psk[:], wk_sb[:, i, :], kt_sb[:, ch, i, :],
                                start=(i == 0), stop=(i == N_DM - 1))
                        kT_writers.append(nc.vector.tensor_scalar_add(
                            kT_loc[:, ch * 512:(ch + 1) * 512], psk[:],
                            b2[:, 1:2]))

                    def scores(kb, nts, kt_buf, kb_off):
                        # kb is the local-first block index into pT
                        for nt in nts:
                            pss = ps_s.tile([128, 512], dt.float32, name="pss",
                                            tag="pss")
                            nc.tensor.matmul(
                                pss[:],
                                kt_buf[:, (kb - kb_off) * 128:
                                       (kb - kb_off + 1) * 128],
                                qT[:, nt, :], start=True, stop=True)
                            nc.scalar.activation(
                                pT[:, kb, nt, :], pss[:],
                                mybir.ActivationFunctionType.Exp, scale=SCALE)

                    # kproj(1) right after the first scores batch (and kt1
                    # streamed before qt1) so the kT send fires ~20us instead
                    # of ~30us -- the partner's half then lands before the
                    # remote-score matmuls reach the head of the PE queue.
                    qproj(0)
                    kproj(0)
                    for kb in range(0, 4):
                        scores(kb, [0], kT_loc, 0)
                    kproj(1)
                    # kb4-7 nt0 before the nt1 batch: their inputs (kT c1,
                    # qT[0]) are ready, so the exp stream doesn't pause while
                    # qproj(1) waits for qt1 to arrive.
                    for kb in range(4, N_KB_H):
                        scores(kb, [0], kT_loc, 0)
                    qproj(1)
                    for kb in range(0, N_KB_H):
                        scores(kb, [1], kT_loc, 0)

                # --- send projected k half to the pair partner ---
                # kT sent in two 512-key chunks on one queue (FIFO ->
                # c0's sem inc lands first). Each chunk fires as soon as its
                # kproj half completes; remote scores on kb8-11 gate only on
                # chunk 0, starting the remote exp stream ~6us earlier.
                prep_ks = []
                for ci in range(2):
                    p = nc.gpsimd.remote_dma_broadcast(
                        kT_rem[:, ci * 512:(ci + 1) * 512],
                        kT_loc[:, ci * 512:(ci + 1) * 512], rs_k, ls,
                        rdests=[(0, 1)] + [None] * 7)
                    bass._add_dep_helper(
                        p.ins, prep_ks[-1].ins if prep_ks else libload.ins,
                        sync=True, reason="ring FIFO order")
                    prep_ks.append(p)
                trig_k = None
                for ci in range(2):
                    t = nc.gpsimd.trigger_dma()
                    bass._add_dep_helper(t.ins, prep_ks[ci].ins, sync=True,
                                         reason="prep before trigger")
                    bass._add_dep_helper(t.ins, kT_writers[ci].ins, sync=True,
                                         reason="kT chunk before send")
                    if trig_k is not None:
                        bass._add_dep_helper(t.ins, trig_k.ins, sync=True,
                                             reason="trigger FIFO order")
                    trig_k = t

                # --- v projection (local half) ---
                with tc.tile_pool(name="ps_v", bufs=2, space="PSUM") as ps_v:
                    v_writers = []
                    for kb in range(N_KB_H):
                        psv = ps_v.tile([128, DV + 1], dt.float32, name="psv",
                                        tag="psv")
                        nc.tensor.matmul(psv[:], ones[:1, :], bva[:1, :],
                                         start=True, stop=False)
                        for i in range(N_DM):
                            nc.tensor.matmul(
                                psv[:, 0:DV],
                                vt_sb[:, kb // 4, i,
                                      (kb % 4) * 128:(kb % 4 + 1) * 128],
                                wv_sb[:, i, :],
                                start=False, stop=(i == N_DM - 1))
                        v_writers.append(
                            nc.vector.tensor_copy(v_loc[:, kb, :], psv[:]))

                    prep_v = nc.gpsimd.remote_dma_broadcast(
                        v_rem[:], v_loc[:], rs_v, ls,
                        rdests=[(0, 1)] + [None] * 7)
                    bass._add_dep_helper(prep_v.ins, prep_ks[1].ins, sync=True,
                                         reason="ring FIFO order")
                    trig_v = nc.gpsimd.trigger_dma()
                    bass._add_dep_helper(trig_v.ins, prep_v.ins, sync=True,
                                         reason="prep before trigger")
                    bass._add_dep_helper(trig_v.ins, trig_k.ins, sync=True,
                                         reason="trigger order")
                    for w in v_writers:
                        bass._add_dep_helper(trig_v.ins, w.ins, sync=True,
                                             reason="v data before send")

                    # --- AV over the local half (k-major accumulate) ---
                    def av(kb, v_buf, kb_off, gate=None):
                        for qb in range(N_QB):
                            mm = nc.tensor.matmul(
                                pso[qb // 3][:, qb % 3, :],
                                pT[:, kb, qb // 4,
                                   (qb % 4) * 128:(qb % 4 + 1) * 128],
                                v_buf[:, kb - kb_off, :],
                                start=(kb == 0 and qb % 3 == 0),
                                stop=(kb == N_KB - 1),
                                skip_group_check=True)
                            if gate is not None:
                                bass._add_dep_helper(mm.ins, gate.ins,
                                                     sync=True,
                                                     reason="remote data gate")

                    # carriers: stall until the partner's halves have landed.
                    # nosync (ordering-only) deps keep the carrier's inline
                    # wait slot free for the post-schedule remote-sem wait,
                    # while still pinning it behind every Vector op our own
                    # sends need (same-engine FIFO = hard ordering), so the
                    # pair cannot deadlock.
                    car_k = nc.vector.memset(car_kt[:], 0.0)
                    for w in kT_writers + v_writers:
                        bass._add_dep_helper(car_k.ins, w.ins, sync=False,
                                             reason="send data first")
                    car_k2 = nc.vector.memset(car_kt2[:], 0.0)
                    bass._add_dep_helper(car_k2.ins, car_k.ins, sync=False,
                                         reason="carrier order")
                    car_v = nc.vector.memset(car_vt[:], 0.0)
                    bass._add_dep_helper(car_v.ins, car_k2.ins, sync=False,
                                         reason="carrier order")

                    def scores_rem(kbs, gate):
                        for kb in kbs:
                            for nt in range(2):
                                pss = ps_s.tile([128, 512], dt.float32,
                                                name="pss", tag="pss")
                                mm = nc.tensor.matmul(
                                    pss[:],
                                    kT_rem[:, (kb - N_KB_H) * 128:
                                           (kb - N_KB_H + 1) * 128],
                                    qT[:, nt, :], start=True, stop=True)
                                bass._add_dep_helper(mm.ins, gate.ins,
                                                     sync=True,
                                                     reason="kT_rem gate")
                                nc.scalar.activation(
                                    pT[:, kb, nt, :], pss[:],
                                    mybir.ActivationFunctionType.Exp,
                                    scale=SCALE)

                    # Remote scores sandwiched between local AV batches: the
                    # ScalarE exp stream starts as soon as the partner's kT
                    # lands, while AV-local batches buffer the PE against a
                    # late carrier.
                    av(0, v_loc, 0)
                    av(1, v_loc, 0)
                    av(2, v_loc, 0)
                    av(3, v_loc, 0)
                    scores_rem(range(8, 12), car_k)
                    av(4, v_loc, 0)
                    av(5, v_loc, 0)
                    scores_rem(range(12, 16), car_k2)
                    av(6, v_loc, 0)
                    av(7, v_loc, 0)
                    for kb in range(N_KB_H, N_KB):
                        av(kb, v_rem, N_KB_H, gate=car_v)

                # --- normalize + output ---
                for qb in range(N_QB):
                    nc.vector.reciprocal(recip[:, qb, :],
                                         pso[qb // 3][:, qb % 3, DV:DV + 1])
                    if qb % 2 == 0:
                        nc.scalar.activation(
                            out_sb[:, qb, :], pso[qb // 3][:, qb % 3, 0:DV],
                            mybir.ActivationFunctionType.Copy,
                            scale=recip[:, qb, :])
                    else:
                        nc.vector.tensor_scalar_mul(
                            out_sb[:, qb, :], pso[qb // 3][:, qb % 3, 0:DV],
                            recip[:, qb, :])
                    if qb % 4 == 3:
                        nc.scalar.dma_start(
                            out_d.ap()[:, qb - 3:qb + 1, :],
                            out_sb[:, qb - 3:qb + 1, :])

    # Tile's scheduling sim can't see the partner's sem increments; attach
    # the real cross-core waits only after scheduling.
    # 8-slot broadcast with one live dest: remote_sem += 16 // 8 = 2
    car_k._wait_ge(rs_k, 2)    # chunk 0 landed
    car_k2._wait_ge(rs_k, 4)   # chunk 1 landed
    car_v._wait_ge(rs_v, 2)
    nc.compile()
    return nc


def kernel(**inputs):
    global _CACHED_NC, LAST_EXEC_NS
    Q = np.asarray(inputs["Q"], dtype=np.float32)
    K = np.asarray(inputs["K"], dtype=np.float32)
    V = np.asarray(inputs["V"], dtype=np.float32)
    WQ = np.asarray(inputs["WQ"], dtype=np.float32)
    bQ = np.asarray(inputs["bQ"], dtype=np.float32)
    WK = np.asarray(inputs["WK"], dtype=np.float32)
    bK = np.asarray(inputs["bK"], dtype=np.float32)
    WV = np.asarray(inputs["WV"], dtype=np.float32)
    bV = np.asarray(inputs["bV"], dtype=np.float32)

    if _CACHED_NC is None:
        _CACHED_NC = _build()
    nc = _CACHED_NC

    def _w(M):  # [dm, dout] -> [128, n_dm, dout]
        return np.ascontiguousarray(
            M.reshape(N_DM, 128, M.shape[1]).transpose(1, 0, 2)).astype(BF16)

    wq = _w(WQ)
    wk = _w(WK)
    wv = _w(WV)
    b2 = np.ascontiguousarray(
        np.stack([bQ, bK], axis=1)).astype(np.float32)  # [DK, 2]
    bva = np.concatenate([bV, np.ones(1, np.float32)]).reshape(1, DV + 1).astype(BF16)

    def _blk(M):  # [1024 seq, dm] -> [2, 128, n_dm, 512] device layout
        return np.ascontiguousarray(
            M.T.reshape(N_DM, 128, 2, 512).transpose(2, 1, 0, 3)).astype(BF16)

    in_maps = []
    for c in range(N_CORES):
        b, h = c // 2, c % 2
        sl = slice(h * LK_H, (h + 1) * LK_H)
        in_maps.append({
            "qt": _blk(Q[b, h * LQ_C:(h + 1) * LQ_C, :]),
            "kt": _blk(K[b, sl, :]),
            "vt": _blk(V[b, sl, :]),
            "wq": wq, "wk": wk, "wv": wv, "b2": b2, "bvaug": bva,
        })

    trace = bool(os.environ.get("KERNEL_TRACE"))
    if trace:
        try:
            import axon_profile_shim  # noqa: F401
        except ImportError:
            trace = False

    res = run_bass_kernel_spmd(nc, in_maps, core_ids=list(range(N_CORES)),
                               trace=trace)
    LAST_EXEC_NS = res.exec_time_ns

    out = np.empty((B, LQ, DV), np.float32)
    for c in range(N_CORES):
        b, h = c // 2, c % 2
        blk = res.results[c]["out"]  # [128, N_QB, DV]
        out[b, h * LQ_C:(h + 1) * LQ_C, :] = (
            blk.transpose(1, 0, 2).reshape(LQ_C, DV))
    return out
